# revision 1
# baseline (speedup 1.0000x reference)
"""Trainium2 Bass kernel for nn_Encoder_78889959293176 (Autoformer-style encoder layer).

Strategy: data-parallel over batch (16 batches -> 8 cores x 2).
All heavy compute on the TensorEngine in a d-major ([channel, time]) layout:
  - QKV projections as W-stationary matmuls
  - autocorrelation statistic mean_value via Q K^T tiles + a 2-copy diagonal
    "shear" DMA into DRAM + ones-matmul partition reduction (flipped-tau space)
  - AllReduce(8 cores) of the batch-summed statistic, on-device top-22 mask
    (iterated max8 + match_replace) and masked softmax -> sparse weight vector g
  - the rolls-weighted aggregation as a circulant matmul against a Toeplitz
    band buffer built from g with a single broadcast DMA (no data-dependent
    indexing anywhere)
  - series decomposition via tensor_tensor_scan cumsum, convs as bf16 matmuls,
    layernorm stats via ones-matmuls.
"""

import numpy as np

import concourse.bass as bass
import concourse.bacc as bacc
import concourse.mybir as mybir
import concourse.tile as tile
from concourse import bass_utils
from concourse.alu_op_type import AluOpType

try:
    import ml_dtypes

    BF16_NP = ml_dtypes.bfloat16
except Exception:  # pragma: no cover
    BF16_NP = np.float32

F32 = mybir.dt.float32
F32R = mybir.dt.float32r
BF16 = mybir.dt.bfloat16
AF = mybir.ActivationFunctionType

B, L, D = 16, 2048, 512
CF = 2048  # conv hidden
TOPK = 22
KER = 25
EPS = 1e-5
SLOPE = 0.01
NCORES = 8
BPC = B // NCORES  # batches per core
DC = D // 128  # 4 d-chunks
CFC = CF // 128  # 16 conv-hidden chunks
TW = L // 512  # 4 time windows of 512
TM = L // 128  # 16 time chunks of 128
NEG = -1.0e30

import os as _os_env

def _kp(name):
    return _os_env.environ.get(name, "1") == "1"


def build(nc: bass.Bass, n_group: int, lite: bool = False):
    x_dm = nc.dram_tensor("x_dm", [BPC, D, L], F32, kind="ExternalInput")
    xb_dm = nc.dram_tensor("x_bf", [BPC, D, L], BF16, kind="ExternalInput")
    wq_d = nc.dram_tensor("wq", [D, D], BF16, kind="ExternalInput")
    wk_d = nc.dram_tensor("wk", [D, D], BF16, kind="ExternalInput")
    wv_d = nc.dram_tensor("wv", [D, D], BF16, kind="ExternalInput")
    wo_d = nc.dram_tensor("wo", [D, D], BF16, kind="ExternalInput")
    bq_d = nc.dram_tensor("bq_t", [128, DC], F32, kind="ExternalInput")
    bk_d = nc.dram_tensor("bk_t", [128, DC], F32, kind="ExternalInput")
    bop_d = nc.dram_tensor("bop_t", [128, DC], F32, kind="ExternalInput")
    w1_d = nc.dram_tensor("w1h", [CFC, DC * 3, 128, 128], BF16, kind="ExternalInput")
    w2_d = nc.dram_tensor("w2h", [DC, CFC * 3, 128, 128], BF16, kind="ExternalInput")
    lng_d = nc.dram_tensor("lng_t", [128, DC], F32, kind="ExternalInput")
    lnb_d = nc.dram_tensor("lnb_t", [128, DC], F32, kind="ExternalInput")
    out_dm = nc.dram_tensor("out_dm", [BPC, D, L], F32, kind="ExternalOutput")

    with tile.TileContext(nc) as tc:
        if lite:
            _body_lite(nc, tc, n_group, x_dm, out_dm)
        else:
            _body(nc, tc, n_group, x_dm, xb_dm, wq_d, wk_d, wv_d, wo_d, bq_d,
                  bk_d, bop_d, w1_d, w2_d, lng_d, lnb_d, out_dm)
    return nc


def _body_lite(nc, tc, n_group, x_dm, out_dm):
    from concourse.alu_op_type import AluOpType as _A

    with (
        tc.tile_pool(name="lsb", bufs=2) as sb,
        tc.tile_pool(name="ldr", bufs=1, space="DRAM") as dr,
    ):
        for b in range(BPC):
            for dc in range(DC):
                t = sb.tile([128, L], F32, tag="t")
                nc.sync.dma_start(t[:], x_dm.ap()[b, 128 * dc : 128 * dc + 128, :])
                nc.sync.dma_start(out_dm.ap()[b, 128 * dc : 128 * dc + 128, :], t[:])
        v = sb.tile([1, L], F32, tag="v")
        nc.vector.memset(v[:], 1.0)
        ci = dr.tile([1, L], F32, tag="ci")
        co = dr.tile([1, L], F32, tag="co")
        nc.sync.dma_start(ci[:], v[:])
        nc.gpsimd.collective_compute(
            "AllReduce", _A.add, replica_groups=[list(range(n_group))],
            ins=[ci[:].opt()], outs=[co[:].opt()],
        )
        nc.sync.dma_start(v[:], co[:])


def _r(ap):
    return ap.bitcast(F32R)


def _load_w_dmajor(nc, dst, src_d):
    nc.sync.dma_start(dst[:], src_d.ap().rearrange("(c p) n -> p c n", p=128))


def _decompose(nc, scan_pool, src, dst, dst_bf16=None):
    """dst[:, dc, 1:L+1] = src - movavg_KER(src) (replicate-padded).
    src: [128, DC, L] f32 tile. dst: [128, DC, L+2] with replicated edge cols.
    Optionally also write a bf16 copy of dst.
    """
    half = (KER - 1) // 2
    for dci in range(DC):
        pad = scan_pool.tile([128, L + KER], F32, tag="scan_pad")
        cs = scan_pool.tile([128, L + KER], F32, tag="scan_cs")
        nc.vector.memset(pad[:, 0:1], 0.0)
        nc.vector.tensor_copy(
            out=pad[:, 1 : 1 + half],
            in_=src[:, dci, 0:1].to_broadcast([128, half]),
        )
        nc.scalar.activation(pad[:, 1 + half : 1 + half + L], src[:, dci, :], AF.Copy)
        nc.vector.tensor_copy(
            out=pad[:, 1 + half + L :],
            in_=src[:, dci, L - 1 : L].to_broadcast([128, half]),
        )
        nc.vector.tensor_tensor_scan(
            out=cs[:], data0=pad[:], data1=pad[:], initial=0.0,
            op0=AluOpType.add, op1=AluOpType.bypass,
        )
        d1 = scan_pool.tile([128, L], F32, tag="scan_d1")
        nc.vector.tensor_sub(out=d1[:], in0=cs[:, KER:], in1=cs[:, 0:L])
        nc.vector.scalar_tensor_tensor(
            out=dst[:, dci, 1 : L + 1], in0=d1[:], scalar=-1.0 / KER,
            in1=src[:, dci, :], op0=AluOpType.mult, op1=AluOpType.add,
        )
        nc.vector.tensor_copy(out=dst[:, dci, 0:1], in_=dst[:, dci, 1:2])
        nc.vector.tensor_copy(
            out=dst[:, dci, L + 1 : L + 2], in_=dst[:, dci, L : L + 1]
        )
        if dst_bf16 is not None:
            nc.vector.tensor_copy(out=dst_bf16[:, dci, :], in_=dst[:, dci, :])


def _body(nc, tc, n_group, x_dm, xb_dm, wq_d, wk_d, wv_d, wo_d, bq_d, bk_d, bop_d,
          w1_d, w2_d, lng_d, lnb_d, out_dm):
    with (
        tc.tile_pool(name="p0", bufs=1) as p0,
        tc.tile_pool(name="pp", bufs=4, space="PSUM") as pp,
        tc.tile_pool(name="dr", bufs=1, space="DRAM") as dr,
        tc.tile_pool(name="dr3", bufs=4, space="DRAM") as dr3,
    ):
        # ----- persistent constants / cross-phase small tiles -----
        ones_mv = p0.tile([128, 1], F32, tag="ones_mv")
        nc.vector.memset(ones_mv[:], 1.0 / D)
        ones_bf = p0.tile([128, 1], BF16, tag="ones_bf")
        nc.vector.memset(ones_bf[:], 1.0 / D)
        bq_c = p0.tile([128, DC], F32, tag="bq_c")
        bk_c = p0.tile([128, DC], F32, tag="bk_c")
        bop_c = p0.tile([128, DC], F32, tag="bop_c")
        lng_c = p0.tile([128, DC], F32, tag="lng_c")
        lnb_c = p0.tile([128, DC], F32, tag="lnb_c")
        nc.sync.dma_start(bq_c[:], bq_d[:, :])
        nc.sync.dma_start(bk_c[:], bk_d[:, :])
        nc.sync.dma_start(bop_c[:], bop_d[:, :])
        nc.sync.dma_start(lng_c[:], lng_d[:, :])
        nc.sync.dma_start(lnb_c[:], lnb_d[:, :])
        hb = []

        # ================= phase 1: mean_value (flipped space) =============
        with tc.tile_pool(name="p12", bufs=1) as p12:
          with (
            tc.tile_pool(name="ph1", bufs=2) as ph1,
            tc.tile_pool(name="ph1b", bufs=2) as ph1b,
            tc.tile_pool(name="ph1w", bufs=3) as ph1w,
            tc.tile_pool(name="ppm1", bufs=1, space="PSUM") as ppm1,
          ):
            mvf = p12.tile([1, BPC * L], F32, tag="mvf")
            wq_s = ph1w.tile([128, DC, D], BF16, tag="wqk")
            wk_s = ph1w.tile([128, DC, D], BF16, tag="wqk2")
            _load_w_dmajor(nc, wq_s, wq_d)
            _load_w_dmajor(nc, wk_s, wk_d)

            for b in range(BPC):
                xb = ph1.tile([128, DC, L], BF16, tag="xb")
                nc.sync.dma_start(
                    xb[:], xb_dm.ap()[b].rearrange("(c p) t -> p c t", p=128)
                )
                q_s = ph1.tile([128, DC, L], BF16, tag="q_s")
                k_s = ph1.tile([128, DC, L], BF16, tag="k_s")
                for w_s, proj, bias in ((wq_s, q_s, bq_c), (wk_s, k_s, bk_c)):
                    for dco in range(DC):
                        for twi in range(TW):
                            ps = pp.tile([128, 512], F32, tag="ps")
                            for dci in range(DC):
                                nc.tensor.matmul(
                                    ps[:],
                                    lhsT=w_s[:, dci, 128 * dco : 128 * dco + 128],
                                    rhs=xb[:, dci, 512 * twi : 512 * twi + 512],
                                    start=(dci == 0), stop=(dci == DC - 1),
                                )
                            nc.scalar.activation(
                                proj[:, dco, 512 * twi : 512 * twi + 512],
                                ps[:], AF.Identity, bias=bias[:, dco : dco + 1],
                            )

                mv_a = ppm1.tile([1, 512], F32, tag="mv0")
                mv_b = ppm1.tile([1, 512], F32, tag="mv1")
                mv_c = ppm1.tile([1, 512], F32, tag="mv2")
                mv_d = ppm1.tile([1, 512], F32, tag="mv3")
                mv_reg = [mv_a, mv_b, mv_c, mv_d]
                def _emit_mv(A, wa):
                    for cc in range(4):
                        w0 = (512 * cc + 128 * A) % L
                        nc.tensor.matmul(
                            mv_reg[cc][0:1, :],
                            lhsT=ones_bf[:],
                            rhs=wa[:, w0 : w0 + 512],
                            start=(A == 0), stop=(A == TM - 1),
                        )

                pend = []
                for A in range(TM if _kp('KP1') else 0):
                    bufA = dr3.tile([128, 4224], BF16, tag="bufA")
                    for tB in range(TW):
                        psc = pp.tile([128, 512], F32, tag="ps")
                        for dci in range(DC):
                            nc.tensor.matmul(
                                psc[:],
                                lhsT=q_s[:, dci, 128 * A : 128 * A + 128],
                                rhs=k_s[:, dci, 512 * tB : 512 * tB + 512],
                                start=(dci == 0), stop=(dci == DC - 1),
                            )
                        c_sb = ph1b.tile([128, 512], BF16, tag="c_sb")
                        nc.scalar.activation(c_sb[:], psc[:], AF.Copy)
                        for cp, eng in ((0, nc.sync), (1, nc.scalar)):
                            dst = bass.AP(
                                bufA[:].tensor,
                                127 + 512 * tB + 2048 * cp,
                                [[4223, 128], [1, 512]],
                            )
                            eng.dma_start(dst, c_sb[:])
                    wa = ph1w.tile([128, 2560], BF16, tag="wa")
                    nc.sync.dma_start(
                        wa[:], bass.AP(bufA[:].tensor, 128, [[4224, 128], [1, 2560]])
                    )
                    pend.append((A, wa))
                    if len(pend) > 2:
                        _emit_mv(*pend.pop(0))
                for a_w in pend:
                    _emit_mv(*a_w)
                for cc in range(4):
                    nc.scalar.activation(
                        mvf[0:1, L * b + 512 * cc : L * b + 512 * cc + 512],
                        mv_reg[cc][0:1, :], AF.Copy,
                    )

          # ================= phase 2: allreduce + topk + softmax =============
          gf_t = p12.tile([1, BPC * L], F32, tag="gf")
          with tc.tile_pool(name="ph2", bufs=1) as ph2:
              lsum = ph2.tile([1, L], F32, tag="sm_tmp")
              nc.vector.tensor_add(out=lsum[:], in0=mvf[0:1, 0:L], in1=mvf[0:1, L : 2 * L])
              cci = dr.tile([1, L], F32, tag="cci")
              cco = dr.tile([1, L], F32, tag="cco")
              nc.sync.dma_start(cci[:], lsum[:])
              _selfcc = _os_env.environ.get("KERNEL_SELFCC", "0") == "1"
              _nocc = _os_env.environ.get("KERNEL_NOCC", "0") == "1"
              if _nocc:
                  nc.sync.dma_start(cco[:], cci[:])
              else:
                  nc.gpsimd.collective_compute(
                      "AllReduce", AluOpType.add,
                      replica_groups=(
                          [[c] for c in range(n_group)] if _selfcc
                          else [list(range(n_group))]
                      ),
                      ins=[cci[:].opt()], outs=[cco[:].opt()],
                  )
              bsum = ph2.tile([1, L], F32, tag="bsum")
              nc.sync.dma_start(bsum[:], cco[:])

              work = ph2.tile([1, L], F32, tag="work")
              mask = ph2.tile([1, L], F32, tag="mask")
              t_on = bsum
              for r, kk in enumerate((8, 8, TOPK - 16)):
                  mx8 = ph2.tile([1, 8], F32, tag=f"mx8_{r}")
                  nc.vector.max(out=mx8[:], in_=t_on[:])
                  if kk < 8:
                      nc.vector.memset(mx8[:, kk:8], NEG)
                  nc.vector.match_replace(
                      out=work[:], in_to_replace=mx8[:], in_values=t_on[:],
                      imm_value=NEG,
                  )
                  t_on = work
              nc.vector.tensor_sub(out=mask[:], in0=bsum[:], in1=work[:])
              nc.vector.tensor_scalar_min(mask[:], mask[:], 1.0)

              for b in range(BPC):
                  gf = gf_t[0:1, L * b : L * b + L]
                  tmp = ph2.tile([1, L], F32, tag="sm_tmp")
                  nc.vector.tensor_mul(out=gf, in0=mvf[0:1, L * b : L * b + L], in1=mask[:])
                  nc.vector.tensor_scalar(
                      out=tmp[:], in0=mask[:], scalar1=1.0, scalar2=1.0e9,
                      op0=AluOpType.subtract, op1=AluOpType.mult,
                  )
                  nc.vector.tensor_add(out=gf, in0=gf, in1=tmp[:])
                  mx = ph2.tile([1, 1], F32, tag="sm_mx")
                  nc.vector.reduce_max(out=mx[:], in_=gf, axis=mybir.AxisListType.X)
                  nc.vector.tensor_scalar_mul(mx[:], mx[:], -1.0)
                  nc.scalar.activation(gf, gf, AF.Exp, bias=mx[:])
                  zz = ph2.tile([1, 1], F32, tag="sm_z")
                  nc.vector.reduce_sum(out=zz[:], in_=gf, axis=mybir.AxisListType.X)
                  nc.vector.reciprocal(out=zz[:], in_=zz[:])
                  nc.vector.tensor_scalar_mul(gf, gf, zz[:])
                  # periodic replication: B[q] = g_f[q mod L]; a row-step-2047
                  # read later yields Gbuf[i, p] = g_f[(127 + p - i) mod L]
                  gfb = ph2.tile([1, L], BF16, tag=f"gfb{b}")
                  nc.vector.tensor_copy(out=gfb[:], in_=gf)
                  hbuf = dr.tile([1, 129 * L], BF16, tag=f"hb{b}")
                  _gs = gfb[:]
                  _ga = [list(p) for p in _gs.ap]
                  grep_ap = bass.AP(
                      _gs.tensor, _gs.offset, [_ga[0], [0, 129], _ga[-1]]
                  )
                  nc.sync.dma_start(
                      hbuf[:].rearrange("a (r n) -> a r n", r=129), grep_ap
                  )
                  hb.append(hbuf)


        # ================= phases 3-7 per batch ============================
        wv_s = p0.tile([128, DC, D], BF16, tag="wvo")
        wo_s = p0.tile([128, DC, D], BF16, tag="wvo2")
        _load_w_dmajor(nc, wv_s, wv_d)
        _load_w_dmajor(nc, wo_s, wo_d)

        for b in range(BPC):
            with tc.tile_pool(name="pyb", bufs=1) as pyb:
              ysb = pyb.tile([128, DC, L], F32, tag="ysb")
              with tc.tile_pool(name="pb", bufs=1) as pb:
                seab = pb.tile([128, DC, L + 2], BF16, tag="seab")

                with tc.tile_pool(name="p34", bufs=1) as p34:
                    acx = p34.tile([128, DC, L], F32, tag="acx")
                    with tc.tile_pool(name="pagg", bufs=1) as pagg:
                        agg = pagg.tile([128, DC, L], BF16, tag="agg")
                        with tc.tile_pool(name="p3v", bufs=1) as p3v:
                            v_s = p3v.tile([128, TM, D], BF16, tag="v_s")
                            with tc.tile_pool(name="p3x", bufs=1) as p3x:
                                xb = p3x.tile([128, DC, L], BF16, tag="xb3")
                                nc.sync.dma_start(
                                    xb[:],
                                    xb_dm.ap()[b].rearrange("(c p) t -> p c t", p=128),
                                )
                                for tm in range(TM if _kp('KP3') else 0):
                                    ps = pp.tile([128, 512], F32, tag="ps")
                                    for dci in range(DC):
                                        nc.tensor.matmul(
                                            ps[:],
                                            lhsT=xb[:, dci, 128 * tm : 128 * tm + 128],
                                            rhs=wv_s[:, dci, :],
                                            start=(dci == 0), stop=(dci == DC - 1),
                                        )
                                    nc.scalar.activation(
                                        v_s[:, tm, :], ps[:], AF.Copy
                                    )

                            gbuf = p3v.tile([128, 3968], BF16, tag="gbuf")
                            nc.sync.dma_start(
                                gbuf[:],
                                bass.AP(hb[b][:].tensor, 127, [[2047, 128], [1, 3968]]),
                            )
                            for dm in range(DC if _kp('KP3') else 0):
                                for nw in range(TW):
                                    ps = pp.tile([128, 512], F32, tag="ps")
                                    for Bc in range(TM):
                                        gp = 512 * nw - 128 * Bc + 1920
                                        nc.tensor.matmul(
                                            ps[:],
                                            lhsT=v_s[:, Bc, 128 * dm : 128 * dm + 128],
                                            rhs=gbuf[:, gp : gp + 512],
                                            start=(Bc == 0), stop=(Bc == TM - 1),
                                        )
                                    nc.scalar.activation(
                                        agg[:, dm, 512 * nw : 512 * nw + 512],
                                        ps[:], AF.Copy,
                                    )

                        with tc.tile_pool(name="p3b", bufs=2) as p3b:
                            for dco in range(DC if _kp('KP3') else 0):
                                for twi in range(TW):
                                    ps = pp.tile([128, 512], F32, tag="ps")
                                    for dci in range(DC):
                                        nc.tensor.matmul(
                                            ps[:],
                                            lhsT=wo_s[:, dci, 128 * dco : 128 * dco + 128],
                                            rhs=agg[:, dci, 512 * twi : 512 * twi + 512],
                                            start=(dci == 0), stop=(dci == DC - 1),
                                        )
                                    xr = p3b.tile([128, 512], F32, tag="xr")
                                    nc.sync.dma_start(
                                        xr[:],
                                        x_dm.ap()[b, 128 * dco : 128 * dco + 128,
                                                  512 * twi : 512 * twi + 512],
                                    )
                                    nc.vector.scalar_tensor_tensor(
                                        out=acx[:, dco, 512 * twi : 512 * twi + 512],
                                        in0=ps[:],
                                        scalar=bop_c[:, dco : dco + 1],
                                        in1=xr[:],
                                        op0=AluOpType.add, op1=AluOpType.add,
                                    )

                    # ---- phase 4: decomposition 1 -> seab (bf16 padded) ----
                    with tc.tile_pool(name="p4", bufs=2) as p4:
                        if _kp('KP4'):
                            _decompose(nc, p4, acx, seab)

                # ---- phase 5: conv1 (bf16) ----
                with tc.tile_pool(name="p56", bufs=1) as p56:
                    h1 = p56.tile([128, CFC, L + 2], BF16, tag="h1")
                    with tc.tile_pool(name="p5", bufs=2) as p5:
                        for co in range(CFC if _kp('KP5') else 0):
                            w1t = p5.tile([128, DC * 3, 128], BF16, tag="w1t")
                            nc.sync.dma_start(
                                w1t[:], w1_d.ap()[co].rearrange("k p n -> p k n")
                            )
                            for nw in range(TW):
                                ps = pp.tile([128, 512], F32, tag="ps")
                                first = True
                                for dci in range(DC):
                                    for tap in range(3):
                                        nc.tensor.matmul(
                                            ps[:],
                                            lhsT=w1t[:, 3 * dci + tap, :],
                                            rhs=seab[:, dci,
                                                     512 * nw + tap : 512 * nw + tap + 512],
                                            start=first,
                                            stop=(dci == DC - 1 and tap == 2),
                                        )
                                        first = False
                                c1t = p5.tile([128, 512], F32, tag="c1t")
                                nc.scalar.activation(c1t[:], ps[:], AF.Copy)
                                nc.vector.scalar_tensor_tensor(
                                    out=h1[:, co, 1 + 512 * nw : 513 + 512 * nw],
                                    in0=c1t[:], scalar=SLOPE, in1=c1t[:],
                                    op0=AluOpType.mult, op1=AluOpType.max,
                                )
                            nc.vector.tensor_copy(
                                out=h1[:, co, 0:1], in_=h1[:, co, 1:2]
                            )
                            nc.vector.tensor_copy(
                                out=h1[:, co, L + 1 : L + 2], in_=h1[:, co, L : L + 1]
                            )

                    # ---- phase 6: conv2 + residual ----
                    if True:
                        with tc.tile_pool(name="p6", bufs=2) as p6:
                            for co in range(DC if _kp('KP6') else 0):
                                w2t = p6.tile([128, CFC * 3, 128], BF16, tag="w2t")
                                nc.sync.dma_start(
                                    w2t[:], w2_d.ap()[co].rearrange("k p n -> p k n")
                                )
                                for nw in range(TW):
                                    ps = pp.tile([128, 512], F32, tag="ps")
                                    first = True
                                    for ci in range(CFC):
                                        for tap in range(3):
                                            nc.tensor.matmul(
                                                ps[:],
                                                lhsT=w2t[:, 3 * ci + tap, :],
                                                rhs=h1[:, ci,
                                                       512 * nw + tap : 512 * nw + tap + 512],
                                                start=first,
                                                stop=(ci == CFC - 1 and tap == 2),
                                            )
                                            first = False
                                    h2t = p6.tile([128, 512], F32, tag="h2t")
                                    nc.scalar.activation(h2t[:], ps[:], AF.Copy)
                                    h2r = p6.tile([128, 512], F32, tag="h2r")
                                    nc.vector.scalar_tensor_tensor(
                                        out=h2r[:], in0=h2t[:], scalar=SLOPE,
                                        in1=h2t[:],
                                        op0=AluOpType.mult, op1=AluOpType.max,
                                    )
                                    nc.vector.tensor_add(
                                        out=ysb[:, co, 512 * nw : 512 * nw + 512],
                                        in0=h2r[:],
                                        in1=seab[:, co, 1 + 512 * nw : 513 + 512 * nw],
                                    )

              # ---- phase 7: decomposition 2 + layernorm (seab/h1 freed) ----
              with (
                  tc.tile_pool(name="p7", bufs=1) as p7,
                  tc.tile_pool(name="p7b", bufs=2) as p7b,
                  tc.tile_pool(name="p7s", bufs=1) as p7s,
                  tc.tile_pool(name="ppm7", bufs=1, space="PSUM") as ppm7,
              ):
                            sea2 = p7.tile([128, DC, L + 2], F32, tag="sea2")
                            if _kp('KP7'):
                                _decompose(nc, p7s, ysb, sea2)
                            mu = p7.tile([1, L], F32, tag="mu")
                            ms = p7.tile([1, L], F32, tag="ms")
                            for twi in range(TW if _kp('KP7') else 0):
                                st_s = ppm7.tile([1, 512], F32, tag="st_s")
                                st_q = ppm7.tile([1, 512], F32, tag="st_q")
                                for dci in range(DC):
                                    sqt = p7b.tile([128, 512], F32, tag="sqt")
                                    nc.scalar.activation(
                                        sqt[:],
                                        sea2[:, dci, 1 + 512 * twi : 513 + 512 * twi],
                                        AF.Square,
                                    )
                                    nc.tensor.matmul(
                                        st_s[0:1, :],
                                        lhsT=ones_mv[:],
                                        rhs=sea2[:, dci, 1 + 512 * twi : 513 + 512 * twi],
                                        start=(dci == 0), stop=(dci == DC - 1),
                                    )
                                    nc.tensor.matmul(
                                        st_q[0:1, :],
                                        lhsT=ones_mv[:],
                                        rhs=sqt[:],
                                        start=(dci == 0), stop=(dci == DC - 1),
                                    )
                                nc.scalar.activation(
                                    mu[0:1, 512 * twi : 512 * twi + 512],
                                    st_s[0:1, :], AF.Copy,
                                )
                                nc.scalar.activation(
                                    ms[0:1, 512 * twi : 512 * twi + 512],
                                    st_q[0:1, :], AF.Copy,
                                )
                            rs = p7.tile([1, L], F32, tag="rs")
                            mub = p7.tile([128, L], F32, tag="mub")
                            rsb = p7.tile([128, L], F32, tag="rsb")
                            if _kp('KP7'):
                                nc.vector.tensor_mul(out=rs[:], in0=mu[:], in1=mu[:])
                                nc.vector.tensor_sub(out=rs[:], in0=ms[:], in1=rs[:])
                                nc.vector.tensor_scalar_add(rs[:], rs[:], EPS)
                                nc.vector.reciprocal(out=rs[:], in_=rs[:])
                                nc.scalar.activation(rs[:], rs[:], AF.Sqrt)
                                mub_d = dr.tile([1, L], F32, tag="mub_d")
                                rsb_d = dr.tile([1, L], F32, tag="rsb_d")
                                nc.sync.dma_start(mub_d[:], mu[:])
                                nc.sync.dma_start(rsb_d[:], rs[:])
                                nc.sync.dma_start(
                                    mub[:],
                                    bass.AP(mub_d[:].tensor, 0, [[0, 128], [1, L]]),
                                )
                                nc.sync.dma_start(
                                    rsb[:],
                                    bass.AP(rsb_d[:].tensor, 0, [[0, 128], [1, L]]),
                                )
                            for dci in range(DC if _kp('KP7') else 0):
                                og = p7b.tile([128, L], F32, tag="og")
                                nc.vector.tensor_sub(
                                    out=og[:], in0=sea2[:, dci, 1 : L + 1], in1=mub[:]
                                )
                                nc.vector.scalar_tensor_tensor(
                                    out=og[:], in0=og[:],
                                    scalar=lng_c[:, dci : dci + 1], in1=rsb[:],
                                    op0=AluOpType.mult, op1=AluOpType.mult,
                                )
                                nc.scalar.activation(
                                    og[:], og[:], AF.Identity,
                                    bias=lnb_c[:, dci : dci + 1],
                                )
                                nc.scalar.dma_start(
                                    out_dm.ap()[b, 128 * dci : 128 * dci + 128, :],
                                    og[:],
                                )


# ---------------------------------------------------------------------------
# host side
# ---------------------------------------------------------------------------
_CACHE = {}


def _get_nc(n_group: int, lite: bool = False):
    key = (n_group, lite)
    if key not in _CACHE:
        nc = bacc.Bacc("TRN2", target_bir_lowering=False, debug=False,
                       num_devices=n_group)
        build(nc, n_group, lite=lite)
        nc.compile()
        _CACHE[key] = nc
    return _CACHE[key]


def stage_inputs(inputs, ncores=NCORES):
    x = np.asarray(inputs["x"], np.float32)
    Wq = np.asarray(inputs["Wq"], np.float32)
    Wk = np.asarray(inputs["Wk"], np.float32)
    Wv = np.asarray(inputs["Wv"], np.float32)
    Wo = np.asarray(inputs["Wo"], np.float32)
    bq = np.asarray(inputs["bq"], np.float32)
    bk = np.asarray(inputs["bk"], np.float32)
    bv = np.asarray(inputs["bv"], np.float32)
    bo = np.asarray(inputs["bo"], np.float32)
    w1 = np.asarray(inputs["conv1_w"], np.float32)
    w2 = np.asarray(inputs["conv2_w"], np.float32)
    lng = np.asarray(inputs["ln_g"], np.float32)
    lnb = np.asarray(inputs["ln_b"], np.float32)

    bop = bo + bv @ Wo
    col = lambda v: np.ascontiguousarray(v.reshape(DC, 128).T)
    w1h = np.ascontiguousarray(
        w1.reshape(3, DC, 128, CFC, 128).transpose(3, 1, 0, 2, 4)
    ).reshape(CFC, DC * 3, 128, 128).astype(BF16_NP)
    w2h = np.ascontiguousarray(
        w2.reshape(3, CFC, 128, DC, 128).transpose(3, 1, 0, 2, 4)
    ).reshape(DC, CFC * 3, 128, 128).astype(BF16_NP)

    shared = {
        "wq": Wq.astype(BF16_NP), "wk": Wk.astype(BF16_NP),
        "wv": Wv.astype(BF16_NP), "wo": Wo.astype(BF16_NP),
        "bq_t": col(bq), "bk_t": col(bk), "bop_t": col(bop),
        "w1h": w1h, "w2h": w2h, "lng_t": col(lng), "lnb_t": col(lnb),
    }
    bpc = B // ncores
    in_maps = []
    for c in range(ncores):
        m = dict(shared)
        xc = np.ascontiguousarray(x[bpc * c : bpc * (c + 1)].transpose(0, 2, 1))
        m["x_dm"] = xc
        m["x_bf"] = xc.astype(BF16_NP)
        in_maps.append(m)
    return in_maps


def kernel(**inputs):
    nc = _get_nc(NCORES)
    in_maps = stage_inputs(inputs)
    res = bass_utils.run_bass_kernel_spmd(nc, in_maps, core_ids=list(range(NCORES)))
    out = np.empty((B, L, D), np.float32)
    for c in range(NCORES):
        o = np.asarray(res.results[c]["out_dm"])  # [BPC, D, L]
        for i in range(BPC):
            out[BPC * c + i] = o[i].T
    return out

